# revision 15
# baseline (speedup 1.0000x reference)
"""Trainium2 Bass kernel for a 2-layer GCN (PyG GCNConv + dense layer).

Computation (matches the jax reference):
    deg[n]  = 1 + sum of incoming edge weights        (self loop weight 1)
    dinv    = deg ** -0.5
    norm_e  = dinv[src] * ew * dinv[dst]              (per edge, incl. self)
    agg[n]  = sum_e norm_e * x[src_e]                 (propagate FIRST: A(xW) == (Ax)W)
    h       = relu(agg @ W1 + b1)
    out     = relu(h @ W2 + b2)

Distribution: nodes (as scatter destinations) are partitioned across the 8
cores.  The host pre-buckets each core's incoming edges into 128-edge chunks
per 128-node destination tile and materializes the edge-ordered source-row
stream x[src_e] (bf16) in exactly the SBUF layout the kernel consumes, so the
device reads it with plain sequential HWDGE DMAs at full HBM bandwidth — no
SWDGE descriptor generation (dma_gather's ~3.5ns/index descriptor-gen on the
gpsimd engine was the previous bottleneck).  Each 128-edge chunk becomes one
bf16 matmul  xg^T @ S  accumulating the feature-major aggregation in PSUM,
where S holds the edge norms at the destination columns.  S matrices for a
whole 4-tile batch are built in two batched DVE tensor_tensor ops (is_equal
against a tiled iota, then scale by the norms).  W1/W2 run as bf16 matmuls
with f32 PSUM accumulation; biases+relu fuse into scalar-engine activations.
The output stays feature-major on device; the host transposes and
un-permutes rows (cheap vs. 52 PE transposes + DVE copies per core).

Host-side work is graph preprocessing only: self-loop append, degree / norm
computation, edge bucketing by destination tile, the bf16 edge stream
gather, and the final row un-permutation of the outputs.
"""

import os
import sys

import numpy as np

sys.path.insert(0, "/opt/trn_rl_repo")

P = 128
N_CORES = 8
DST_T = int(os.environ.get("GCN_DST_T", "64"))   # nodes per destination tile
TPB = 512 // DST_T    # tiles per 512-node batch (double buffered)

D_IN = 128
D_HID = 512
D_OUT = 128


def _greedy_tiles(cnt, n_tiles):
    """Assign local nodes to n_tiles bins of <=DST_T nodes, balancing incoming
    edge counts (the max per-tile count drives the padded chunk count K for
    every tile on every core).  Returns tile_of[node], pos_in_tile[node]."""
    n = len(cnt)
    order = np.argsort(-cnt, kind="stable")
    tile_of = np.empty(n, np.int32)
    pos_in_tile = np.empty(n, np.int32)
    counts = np.zeros(n_tiles, np.int32)
    load = np.zeros(n_tiles, np.int64)
    big = np.int64(1 << 60)
    for node in order:
        score = np.where(counts < DST_T, load + cnt[node], big)
        t = int(np.argmin(score))
        tile_of[node] = t
        pos_in_tile[node] = counts[t]
        counts[t] += 1
        load[t] += cnt[node]
    return tile_of, pos_in_tile


def _schunks(bK):
    """Split a batch's bK slots into even-sized chunks (the local_scatter
    GPSIMD-RAM limit is num_elems*32 < 2**16 elems; num_idxs must be even)."""
    cap = (2047 // DST_T) & ~1
    out, off = [], 0
    while bK - off > cap:
        out.append((off, cap))
        off += cap
    if bK - off:
        out.append((off, bK - off))
    return out


def _preprocess(x, edge_index, edge_weight):
    """Full-graph preprocessing; returns per-core packed arrays + layout."""
    N = x.shape[0]
    n_per = N // N_CORES
    assert n_per * N_CORES == N

    src = np.asarray(edge_index[0], np.int64)
    dst = np.asarray(edge_index[1], np.int64)
    ew = np.asarray(edge_weight, np.float32)
    ids = np.arange(N, dtype=np.int64)
    src_f = np.concatenate([src, ids])
    dst_f = np.concatenate([dst, ids])
    ew_f = np.concatenate([ew, np.ones(N, np.float32)])

    deg = np.bincount(dst_f, weights=ew_f.astype(np.float64), minlength=N)
    deg = deg.astype(np.float32)
    dinv = np.where(deg > 0, 1.0 / np.sqrt(deg), 0.0).astype(np.float32)
    norm = (ew_f * dinv[src_f] * dinv[dst_f]).astype(np.float32)

    n_tiles = -(-n_per // DST_T)          # real tiles per core
    n_batches = -(-n_tiles // TPB)
    tiles_tot = n_batches * TPB           # padded tile count (ghost tiles)

    cores = []
    for c in range(N_CORES):
        lo, hi = c * n_per, (c + 1) * n_per
        m = (dst_f >= lo) & (dst_f < hi)
        es = src_f[m]
        ed = (dst_f[m] - lo).astype(np.int64)
        en = norm[m]
        cnt = np.bincount(ed, minlength=n_per)
        # pack real nodes into the first n_tiles bins only, so trailing
        # ghost tiles are empty and their matmuls can be skipped
        tile_of, pos_in_tile = _greedy_tiles(cnt, n_tiles)

        te = tile_of[ed]
        order = np.argsort(te, kind="stable")
        es, ed, en, te = es[order], ed[order], en[order], te[order]
        seg_starts = np.searchsorted(te, np.arange(tiles_tot), side="left")
        rank = np.arange(len(es)) - seg_starts[te]
        tile_len = np.bincount(te, minlength=tiles_tot)

        cores.append(dict(es=es, en=en, ed=ed, te=te, rank=rank,
                          tile_len=tile_len, tile_of=tile_of,
                          pos_in_tile=pos_in_tile, lo=lo))

    K = max(1, int(max(-(-core["tile_len"].max() // P) for core in cores)))
    n_slots = tiles_tot * K

    bK = TPB * K
    chunks = _schunks(bK)
    # chunk-local slot offset for every slot (same for every batch)
    sloc = np.empty(bK, np.int64)
    for off, csz in chunks:
        sloc[off:off + csz] = np.arange(csz)
    sloc_all = np.tile(sloc, n_slots // bK)

    per_core = []
    for core in cores:
        src_lin = np.zeros(n_slots * P, np.int64)
        mnorm = np.zeros(n_slots * P, np.float32)
        sidx = np.full(n_slots * P, -1, np.int16)

        slot = core["te"] * K + core["rank"] // P
        lin = slot * P + core["rank"] % P
        src_lin[lin] = core["es"]
        mnorm[lin] = core["en"]
        sidx[lin] = (sloc_all[slot] * DST_T
                     + core["pos_in_tile"][core["ed"]]).astype(np.int16)

        # permutation: tile-slot row -> global node id (-1 for ghosts)
        perm = np.full(tiles_tot * DST_T, -1, np.int64)
        node_rows = (core["tile_of"].astype(np.int64) * DST_T
                     + core["pos_in_tile"])
        perm[node_rows] = np.arange(len(core["tile_of"])) + core["lo"]

        per_core.append(dict(
            src_lin=src_lin,
            mnorm=mnorm.reshape(n_slots, P).T.copy(),   # [128, n_slots]
            sidx=sidx.reshape(n_slots, P).T.copy(),
            perm=perm,
        ))

    layout = dict(K=K, n_slots=n_slots, n_batches=n_batches,
                  tiles_tot=tiles_tot, n_tiles_real=n_tiles)
    return per_core, layout


def _build_program(layout):
    from concourse import bacc, mybir, tile

    f32 = mybir.dt.float32
    bf16 = mybir.dt.bfloat16
    K = layout["K"]
    n_batches = layout["n_batches"]
    n_slots = layout["n_slots"]
    tiles_tot = layout["tiles_tot"]
    out_cols = tiles_tot * DST_T
    bK = TPB * K                      # slots per batch

    # cdata (f32): b1c(4) | b2c(1)
    O_B1, O_B2 = 0, 4
    C_COLS = 5
    # cdata16 (bf16): w1(512) | w2r(512) | mnorm(n_slots)
    H_W1, H_W2, H_MNORM = 0, 512, 1024
    H_COLS = H_MNORM + n_slots
    chunks = _schunks(bK)

    nc = bacc.Bacc("TRN2")
    xs_d = nc.declare_dram_parameter("xs", [P, n_slots, D_IN], bf16,
                                     isOutput=False)
    cdata_d = nc.declare_dram_parameter("cdata", [P, C_COLS], f32,
                                        isOutput=False)
    cdata16_d = nc.declare_dram_parameter("cdata16", [P, H_COLS], bf16,
                                          isOutput=False)
    i16 = mybir.dt.int16
    sidx_d = nc.declare_dram_parameter("sidx", [P, n_slots], i16,
                                       isOutput=False)
    out_d = nc.declare_dram_parameter("out", [P, out_cols], bf16,
                                      isOutput=True)

    with tile.TileContext(nc) as tc:
        with (
            tc.tile_pool(name="const", bufs=1) as const,
            tc.tile_pool(name="gbuf", bufs=4) as gbuf,
            tc.tile_pool(name="spool", bufs=3) as spool,
            tc.tile_pool(name="aggp", bufs=3) as aggp,
            tc.tile_pool(name="hp", bufs=3) as hp,
            tc.tile_pool(name="outp", bufs=3) as outp,
            tc.tile_pool(name="psa", bufs=2, space="PSUM") as psa,
            tc.tile_pool(name="psh", bufs=2, space="PSUM") as psh,
            tc.tile_pool(name="pso", bufs=2, space="PSUM") as pso,
        ):
            # ---- constants ----
            cdata_s = const.tile([P, C_COLS], f32)
            nc.scalar.dma_start(out=cdata_s[:], in_=cdata_d[:])
            cdata16_s = const.tile([P, H_COLS], bf16)
            nc.scalar.dma_start(out=cdata16_s[:], in_=cdata16_d[:])
            sidx_s = const.tile([P, n_slots], i16)
            nc.scalar.dma_start(out=sidx_s[:], in_=sidx_d[:])

            def w1_sl(cc):
                return cdata16_s[:, H_W1 + cc * P:H_W1 + (cc + 1) * P]

            def w2_sl(cc):
                return cdata16_s[:, H_W2 + cc * P:H_W2 + (cc + 1) * P]

            def b1_sl(cc):
                return cdata_s[:, O_B1 + cc:O_B1 + cc + 1]

            b2_sl = cdata_s[:, O_B2:O_B2 + 1]
            mnorm_s = cdata16_s[:, H_MNORM:H_MNORM + n_slots]

            relu = mybir.ActivationFunctionType.Relu

            n_tiles_real = layout["n_tiles_real"]
            for g in range(n_batches):
                n_rt = max(0, min(TPB, n_tiles_real - g * TPB))
                nu = n_rt * K               # slots actually consumed
                nu_ev = min(bK, nu + (nu & 1))
                c0 = g * bK
                xsb = gbuf.tile([P, bK, D_IN], bf16, tag="xs")
                nc.sync.dma_start(
                    out=xsb[:, :nu, :], in_=xs_d[:, c0:c0 + nu, :])

                # S build on gpsimd: zero + scatter the edge norms to
                # their destination columns, one call per slot chunk
                Sb = spool.tile([P, bK, DST_T], bf16, tag="S")
                for cs, csz in _schunks(nu_ev):
                    nc.gpsimd.local_scatter(
                        out_ap=Sb[:, cs:cs + csz, :],
                        data_ap=mnorm_s[:, c0 + cs:c0 + cs + csz],
                        idxs_ap=sidx_s[:, c0 + cs:c0 + cs + csz],
                        channels=P, num_elems=csz * DST_T, num_idxs=csz)

                pagg = psa.tile([P, TPB * DST_T], f32, space="PSUM")
                if n_rt < TPB:
                    # ghost-tile columns get no matmuls; init them so the
                    # group-wide eviction reads defined data
                    nc.vector.memset(pagg[:, n_rt * DST_T:], 0)
                for tb in range(n_rt):
                    for j in range(K):
                        sl = tb * K + j
                        nc.tensor.matmul(
                            out=pagg[:, tb * DST_T:(tb + 1) * DST_T],
                            lhsT=xsb[:, sl, :],
                            rhs=Sb[:, sl, :],
                            start=(j == 0),
                            stop=(j == K - 1),
                        )

                aggT = aggp.tile([P, TPB * DST_T], bf16)
                nc.vector.tensor_copy(out=aggT[:], in_=pagg[:])

                # layer 1: hT[c] = relu(W1c^T @ aggT + b1c)
                hT = hp.tile([P, 4, TPB * DST_T], bf16)
                for cc in range(4):
                    ph = psh.tile([P, TPB * DST_T], f32, space="PSUM")
                    nc.tensor.matmul(
                        out=ph[:],
                        lhsT=w1_sl(cc),
                        rhs=aggT[:],
                        start=True, stop=True,
                    )
                    nc.scalar.activation(
                        out=hT[:, cc, :], in_=ph[:], func=relu,
                        bias=b1_sl(cc), scale=1.0,
                    )

                # layer 2: outT = relu(sum_c W2c^T @ hT[c] + b2)
                po = pso.tile([P, TPB * DST_T], f32, space="PSUM")
                for cc in range(4):
                    nc.tensor.matmul(
                        out=po[:],
                        lhsT=w2_sl(cc),
                        rhs=hT[:, cc, :],
                        start=(cc == 0), stop=(cc == 3),
                    )
                outT = outp.tile([P, TPB * DST_T], bf16, tag="outT")
                nc.scalar.activation(
                    out=outT[:], in_=po[:], func=relu,
                    bias=b2_sl, scale=1.0,
                )

                # feature-major output; the host transposes + un-permutes
                nc.scalar.dma_start(
                    out=out_d[:, g * TPB * DST_T:(g + 1) * TPB * DST_T],
                    in_=outT[:])

    nc.compile()
    return nc


def _install_ntff_hook():
    """The agent image's antenv lacks axon_hooks; fabricate it so trace=True
    can drive NTFF profiling through libaxon_pjrt.so's C ABI."""
    import contextlib
    import ctypes
    import types

    if "antenv.axon_hooks" in sys.modules:
        return
    so_path = "/opt/axon/libaxon_pjrt.so"
    if not os.path.exists(so_path):
        return
    lib = ctypes.CDLL(so_path)
    if not hasattr(lib, "axon_start_nrt_profile"):
        return
    lib.axon_start_nrt_profile.argtypes = [
        ctypes.POINTER(ctypes.c_int64), ctypes.c_size_t]
    lib.axon_start_nrt_profile.restype = ctypes.c_int64
    lib.axon_stop_nrt_profile.argtypes = [ctypes.c_char_p]
    lib.axon_stop_nrt_profile.restype = ctypes.c_int64

    @contextlib.contextmanager
    def _hook(output_dir, device_ids):
        import jax
        jax.devices()
        if device_ids:
            ids = (ctypes.c_int64 * len(device_ids))(*device_ids)
            rc = lib.axon_start_nrt_profile(ids, len(device_ids))
        else:
            rc = lib.axon_start_nrt_profile(None, 0)
        if rc != 0:
            raise RuntimeError(f"axon_start_nrt_profile rc={rc}")
        try:
            yield
        finally:
            n = lib.axon_stop_nrt_profile(str(output_dir).encode())
            print(f"ntff profile: {n} file(s) written to {output_dir}",
                  file=sys.stderr)

    import antenv  # noqa: F401
    mod = types.ModuleType("antenv.axon_hooks")
    mod._hook = _hook
    mod.set_axon_ntff_profile_hook = lambda h: setattr(mod, "_hook", h)
    mod.get_axon_ntff_profile_hook = lambda: mod._hook
    sys.modules["antenv.axon_hooks"] = mod


def _assemble_inputs(x, W1, b1, W2, b2, per_core, layout):
    import ml_dtypes

    w2r = W2.reshape(4, P, D_OUT).transpose(1, 0, 2).reshape(P, 4 * D_OUT)
    b1c = b1.reshape(4, P).T
    b2c = b2.reshape(P, 1)

    x16 = x.astype(ml_dtypes.bfloat16)
    n_slots = layout["n_slots"]

    in_maps = []
    for pc in per_core:
        # edge-ordered source-row stream in SBUF layout [128, n_slots, 128]
        xs = x16[pc["src_lin"]].reshape(n_slots, P, D_IN).transpose(1, 0, 2)
        cdata = np.concatenate([b1c, b2c], axis=1).astype(np.float32)
        cdata16 = np.concatenate(
            [W1, w2r, pc["mnorm"]], axis=1).astype(ml_dtypes.bfloat16)
        in_maps.append({
            "xs": np.ascontiguousarray(xs),
            "cdata": np.ascontiguousarray(cdata),
            "cdata16": np.ascontiguousarray(cdata16),
            "sidx": np.ascontiguousarray(pc["sidx"]),
        })
    return in_maps


def _run(nc, in_maps, trace=False):
    if trace:
        try:
            _install_ntff_hook()
        except Exception as e:  # degrade to untraced run
            print(f"ntff hook install failed: {e}", file=sys.stderr)
    from concourse.bass_utils import run_bass_kernel_spmd

    return run_bass_kernel_spmd(
        nc, in_maps, core_ids=list(range(N_CORES)), trace=trace,
    )


def kernel(x, edge_index, edge_weight, W1, b1, W2, b2, _want_trace=False):
    x = np.ascontiguousarray(np.asarray(x, np.float32))
    W1 = np.asarray(W1, np.float32)
    b1 = np.asarray(b1, np.float32)
    W2 = np.asarray(W2, np.float32)
    b2 = np.asarray(b2, np.float32)

    N = x.shape[0]
    per_core, layout = _preprocess(x, edge_index, edge_weight)
    nc = _build_program(layout)

    in_maps = _assemble_inputs(x, W1, b1, W2, b2, per_core, layout)
    res = _run(nc, in_maps, trace=_want_trace)

    out = np.empty((N, D_IN), np.float32)
    for c in range(N_CORES):
        rows = np.asarray(res.results[c]["out"]).astype(np.float32).T
        perm = per_core[c]["perm"]
        valid = perm >= 0
        out[perm[valid]] = rows[valid]

    kernel.last_results = res
    return out
